# revision 1
# baseline (speedup 1.0000x reference)
"""Trainium2 Bass kernel for the 1x1-conv attention module (fp8 pipeline).

Shapes (hardcoded): x (8, 64, 64, 64) fp32, w_qkv (192, 64), b_qkv (192,),
w_out (64, 64), b_out (64,). Data-parallel: one batch element per NeuronCore
(8 cores). Channel-major everywhere (c on partitions, t = h*64+w on the free
dim); the reference's view/permute quirk composes to the standard
channel-major permute, so no data movement is needed for it.

Numerics/HW design (validated by on-device probes):
  Both big matmuls run as fp8e4m3 DoubleRow (0.5 cyc/row, 2x fp32r peak).
  QK packs a 3-term split-precision product into the 256-slot DoubleRow
  contraction: (p 0-63, s0) kh*qh, (p 64-127, s0) kl*qh, (p 0-63, s1)
  kh*ql, plus a (row 64, s1) bias channel that adds +2.0 to every score
  so the exp encodings below stay in range. kl/ql are fp8 residuals
  f8(k - f8(k)), which cancel the dominant q/k quantization error
  (rel err 1.4e-2 -> 5.6e-3, vs the 2e-2 gate).
  Scores (PSUM fp32) turn into fp8 softmax weights e = e^s/128 on TWO
  engines in parallel -- the exp is the serial bottleneck of the whole
  kernel (131k lane-cycles/core vs the PE's ~120k):
    Act: native table exp with bias -ln(128), fp8e4 output (exact e4m3
         rounding, probe-verified);
    DVE: Schraudolph in log2 space -- t = min(s*8/ln2, 119) converted to
         uint8 (saturating at 0, probe-verified) IS the e4m3 bit pattern
         of 2^(t/8-7) ~= e^s/128.  At fp8 output precision this equals
         rounding true exp to the e4m3 grid, so one tensor_scalar per
         tile replaces exp.  119 caps at e4m3 max-normal 240 (code 120
         is inf: HW fp8e4 is the IEEE e4m3 variant).
  PV pairs two j-chunks per DoubleRow matmul: vt [128, 2, 96] stationary
  (v + ones column for the softmax row-sums + zero pad: dual-fp8
  ldweights requires a multiple-of-32 column count), e3 [128, 2, 512]
  moving -- 256 cyc per 2 j-chunks, 4x fp32r.
  Normalization lags one i-chunk: drain u (Act), reciprocal of rowsums
  (DVE), K=1 broadcast matmul (PE), u*rinv (DVE), output projection (PE),
  +residual (DVE), DMA out.  PV emission is deferred `defer` pairs behind
  QK so the in-order PE queue never parks on an unfinished exp.
"""

import numpy as np

B, C, HW = 8, 64, 4096
NCORES = 8
IC = 512            # i-chunk (query tokens per block)
NIC = HW // IC      # 8
NJ = HW // 128      # 32 j-chunks of 128 tokens
NP = NJ // 2        # 16 j-chunk pairs
A_SCH = 11.541560327111707      # 8/ln2
SBIAS = 2.0                     # score offset added via the QK bias channel
ACT_BIAS = -4.852030263919617   # -ln(128): Act exp output = e^s/128
TCLAMP = 119.0                  # e4m3 max normal (240); 120 encodes inf

_compiled = None


def _build_bass(repeat=1, dve_n=8, defer=2, epool_bufs=5, spool_bufs=3,
                norm_slots=(2, 3, 6, 7, 8), dve_pairs=None, conv_dve=0,
                do_exp=True, do_av=True, do_norm=True):
    import concourse.bass as bass
    import concourse.mybir as mybir
    import concourse.tile as tile

    FP = mybir.dt.float32
    FR = mybir.dt.float32r
    F8 = mybir.dt.float8e4
    U8 = mybir.dt.uint8
    Exp = mybir.ActivationFunctionType.Exp
    DR = mybir.MatmulPerfMode.DoubleRow
    Mult = mybir.AluOpType.mult
    Min = mybir.AluOpType.min

    # DVE handles dve_n of the NP exp pairs per i-chunk (rest on Act).
    # Pair 15 goes to DVE so the next chunk's reciprocal (gated on PV 15,
    # gated on exp 15) follows DVE's own queue without an idle bubble.
    if dve_pairs is not None:
        dve_set = set(dve_pairs)
    elif dve_n:
        dve_set = {int(2 + i * NP / dve_n) % NP for i in range(dve_n)} | {15}
        while len(dve_set) > dve_n:
            dve_set.remove(min(dve_set))
    else:
        dve_set = set()

    nc = bass.Bass("TRN2", target_bir_lowering=False, debug=False)

    xa_d = nc.dram_tensor("xa", [C + 1, HW], FP, kind="ExternalInput")
    xb_d = nc.dram_tensor("xb", [C, HW], FP, kind="ExternalInput")
    wq_d = nc.dram_tensor("wq", [C + 1, C], FP, kind="ExternalInput")
    wk_d = nc.dram_tensor("wk", [C + 1, C], FP, kind="ExternalInput")
    wv_d = nc.dram_tensor("wv", [C + 1, C], FP, kind="ExternalInput")
    wo_d = nc.dram_tensor("wo", [C, C], FP, kind="ExternalInput")
    cq1_d = nc.dram_tensor("cq1", [128, HW], F8, kind="ExternalInput")
    ck1_d = nc.dram_tensor("ck1", [128, HW], F8, kind="ExternalInput")
    out_d = nc.dram_tensor("out", [C, HW], FP, kind="ExternalOutput")

    with tile.TileContext(nc) as tc:
        with (
            nc.allow_low_precision(reason="fp8 attention (fp32 PSUM accum)"),
            tc.tile_pool(name="singles", bufs=1) as singles,
            tc.tile_pool(name="escr", bufs=epool_bufs) as epool,
            tc.tile_pool(name="usb", bufs=2) as uspool,
            tc.tile_pool(name="attp", bufs=2) as apool,
            tc.tile_pool(name="outp", bufs=2) as opool,
            tc.tile_pool(name="klst", bufs=2) as klpool,
            tc.tile_pool(name="rinvp", bufs=2) as rivpool,
            tc.tile_pool(name="sps", bufs=spool_bufs, space="PSUM") as spool,
            tc.tile_pool(name="ups", bufs=2, space="PSUM") as upool,
        ):
            # ---- load inputs ----
            xa = singles.tile([C + 1, HW], FP)
            xb = singles.tile([C, HW], FP)
            wq = singles.tile([C + 1, C], FP)
            wk = singles.tile([C + 1, C], FP)
            wv = singles.tile([C + 1, C], FP)
            wo = singles.tile([C, C], FP)
            # persistent fp8 operand tensors
            qd = singles.tile([128, 2, HW], F8)
            kd = singles.tile([128, 2, HW], F8)
            vt = singles.tile([128, NP, 2, 96], F8)

            nc.sync.dma_start(out=wq[:], in_=wq_d[:])
            nc.sync.dma_start(out=wk[:], in_=wk_d[:])
            nc.sync.dma_start(out=wv[:], in_=wv_d[:])
            nc.sync.dma_start(out=wo[:], in_=wo_d[:])
            nc.sync.dma_start(out=xa[:], in_=xa_d[:])
            # slot-1 constants (bias channel row 64 + zero rows 65-127);
            # rows 0-63 of slot 1 are overwritten by ql / kh-dup below
            nc.sync.dma_start(out=qd[:, 1, :], in_=cq1_d[:])
            nc.sync.dma_start(out=kd[:, 1, :], in_=ck1_d[:])
            nc.sync.dma_start(out=xb[:], in_=xb_d[:])

            # fp32r copies (walrus wants engine-produced fp32r matmul inputs)
            xar = singles.tile([C + 1, HW], FR)
            wqr = singles.tile([C + 1, C], FR)
            wkr = singles.tile([C + 1, C], FR)
            wvr = singles.tile([C + 1, C], FR)
            wor = singles.tile([C, C], FR)
            nc.vector.tensor_copy(xar[:], xa[:])
            nc.vector.tensor_copy(wqr[:], wq[:])
            nc.vector.tensor_copy(wkr[:], wk[:])
            nc.vector.tensor_copy(wvr[:], wv[:])
            nc.vector.tensor_copy(wor[:], wo[:])

            ones32 = singles.tile([128, 1], FP)
            nc.vector.memset(ones32[:], 1.0)
            bias_act = singles.tile([128, 1], FP)
            nc.vector.memset(bias_act[:], ACT_BIAS)
            # preload the exp table set while DMAs are in flight
            expwarm = singles.tile([1, 1], FP)
            nc.scalar.activation(expwarm[:], ones32[0:1, :], Exp,
                                 bias=bias_act[0:1, :])
            ones_b = singles.tile([1, C], FR)  # K=1 stationary for broadcast
            nc.vector.tensor_copy(
                ones_b[:], ones32[0:1, 0:1].to_broadcast([1, C])
            )
            # vt ones column (row-sums) + zero pad, on the otherwise-idle Pool
            nc.gpsimd.memset(vt[:, :, :, C: C + 1], 1.0)
            nc.gpsimd.memset(vt[:, :, :, C + 1:], 0.0)

            def kd_sl(jc):
                return kd[:, :, jc * 128:(jc + 1) * 128]

            def qd_sl(ic):
                return qd[:, :, ic * IC:(ic + 1) * IC]

            def proj_qk(n, which):
                # projection PSUM comes from the score pool (a [C, IC] fp32
                # tile fits a score slot) so rpool stays 1 bank
                isl = slice(n * IC, (n + 1) * IC)
                pp = spool.tile([C, IC], FP, tag="scores", name=f"prj{which}")
                w_ = wqr if which == "q" else wkr
                nc.tensor.matmul(pp[:], w_[:], xar[:, isl],
                                 start=True, stop=True)
                hconv = (nc.vector.tensor_copy if conv_dve
                         else nc.scalar.copy)
                if which == "q":
                    hconv(qd[0:C, 0, isl], pp[:])                   # qh
                    nc.vector.tensor_sub(qd[0:C, 1, isl], pp[:],
                                         qd[0:C, 0, isl])           # ql
                    nc.sync.dma_start(out=qd[C:128, 0, isl],
                                      in_=qd[0:C, 0, isl])          # qh dup
                else:
                    hconv(kd[0:C, 0, isl], pp[:])                   # kh
                    kls = klpool.tile([C, IC], F8)
                    nc.vector.tensor_sub(kls[:], pp[:], kd[0:C, 0, isl])
                    nc.sync.dma_start(out=kd[0:C, 1, isl],
                                      in_=kd[0:C, 0, isl])          # kh dup
                    nc.sync.dma_start(out=kd[C:128, 0, isl], in_=kls[:])

            def proj_v(g):
                # 4 token-chunks (= vt pairs 2g, 2g+1) per PSUM tile
                pv = spool.tile([128, 4, C], FP, tag="scores", name="pvj")
                for t in range(4):
                    jc = 4 * g + t
                    jsl = slice(jc * 128, (jc + 1) * 128)
                    nc.tensor.matmul(pv[:, t, :], xar[:, jsl], wvr[:],
                                     start=True, stop=True)
                for t in range(2):
                    # on DVE: Act is the busier exp engine
                    nc.vector.tensor_copy(
                        vt[:, 2 * g + t, :, 0:C],
                        pv[:, 2 * t:2 * t + 2, :],
                    )

            pend = []        # deferred PV work: (u, e3, ic, m)
            norm_q = []      # lagged normalization steps for a finished chunk

            def emit_pv(u, e3, ic, m):
                if not do_av:
                    return
                nc.tensor.matmul(
                    u[:], vt[:, m, :, :], e3[:],
                    start=(m == 0), stop=(m == NP - 1),
                    perf_mode=DR,
                )
                if m == NP - 1 and do_norm:
                    norm_q.extend(_norm_steps(ic, u))

            def _norm_steps(pic, pu):
                # u stays in PSUM until att; the row-sum reciprocal is
                # broadcast across partitions by a stride-0 DMA, so the
                # normalization needs NO u drain, NO broadcast matmul, and
                # no Act-engine work at all.
                isl = slice(pic * IC, (pic + 1) * IC)

                rinv = rivpool.tile([1, IC], FR)
                rinv64 = apool.tile([C, IC], FR, name="rinv64")
                att = apool.tile([C, IC], FR)
                o = opool.tile([C, IC], FP)

                def s_recip():
                    nc.vector.reciprocal(rinv[:], pu[C:C + 1, :])
                def s_bcast():
                    nc.sync.dma_start(
                        out=rinv64[:],
                        in_=rinv[0:1, :].unsqueeze(1).to_broadcast([1, C, IC]),
                    )
                def s_att():
                    nc.vector.tensor_mul(att[:], pu[0:C, :], rinv64[:])
                p = [None]
                def s_proj():
                    p[0] = upool.tile([C, IC], FP, tag="u", name="po")
                    nc.tensor.matmul(p[0][:], wor[:], att[:],
                                     start=True, stop=True)
                def s_out():
                    nc.vector.tensor_add(o[:], p[0][:], xb[:, isl])
                    nc.sync.dma_start(out=out_d[:, isl], in_=o[:])

                return [s_recip, s_bcast, s_att, s_proj, s_out]

            # ---- main loop ----
            # Per i-chunk: 16 QK pairs (2 matmuls each into one 2-bank PSUM
            # pair tile), exp per pair on Act or DVE, PV per pair deferred
            # `defer` pairs so the in-order PE queue never parks on an
            # unfinished exp.  3 pair slots = 2 being exp'd (one per
            # engine) + 1 being refilled by the PE: exp engines never
            # starve.  Norm steps of the previous chunk are spread across
            # slots so each sits deep in its engine queue by the time its
            # cross-engine dep resolves.  k/v/q projections for the next
            # repeat stream through the last chunk's slots (rep 0 projects
            # up front; that startup is outside the marginal measurement).
            for rep in range(repeat):
                if rep == 0:
                    proj_qk(0, "q")
                    for n in range(NIC):
                        proj_qk(n, "k")
                    for g in range(NJ // 4):
                        proj_v(g)
                for ic in range(NIC):
                    u = upool.tile([96, IC], FP, tag="u")
                    for m in range(NP):
                        gm = (rep * NIC + ic) * NP + m
                        s2 = spool.tile([128, 2, IC], FP, tag="scores")
                        for half in range(2):
                            nc.tensor.matmul(
                                s2[:, half, :], kd_sl(2 * m + half),
                                qd_sl(ic),
                                start=True, stop=True, perf_mode=DR,
                            )
                        e3 = epool.tile([128, 2, IC], F8)
                        if do_exp:
                            if m in dve_set:
                                nc.vector.tensor_scalar(
                                    e3[:].bitcast(U8), s2[:],
                                    A_SCH, TCLAMP, Mult, Min,
                                )
                            else:
                                nc.scalar.activation(
                                    e3[:], s2[:], Exp, bias=bias_act[:],
                                )
                        pend.append((u, e3, ic, m, gm))
                        # hold cross-chunk PVs until slot 2 so the new
                        # chunk's first QKs aren't queued behind them
                        pops = 0
                        while (pend and pend[0][4] + defer <= gm
                               and m >= 2 and pops < 2):
                            emit_pv(*pend.pop(0)[:4])
                            pops += 1
                        # lagged normalization of the previous i-chunk
                        if norm_q and m in norm_slots:
                            norm_q.pop(0)()
                        # stream the next i-chunk's q projection
                        if m == 12:
                            if not (rep == repeat - 1 and ic == NIC - 1):
                                proj_qk((ic + 1) % NIC, "q")
                        # stream the next repeat's k/v through the last
                        # chunk (safe: kd chunk n's last read is QK pair
                        # 2n+1; vt pair 2g+1's last read is its chunk-7 PV)
                        if ic == NIC - 1 and rep < repeat - 1:
                            if m >= 2 and m % 2 == 0:
                                proj_qk((m - 2) // 2, "k")
                            if m >= 4 and m % 2 == 0:
                                proj_v((m - 4) // 2)
                        if ic == 0 and rep > 0:
                            if m == 0:
                                proj_qk(7, "k")
                            elif m == 1:
                                proj_v(6)
                            elif m == 2:
                                proj_v(7)

            # ---- tail ----
            for w in pend:
                emit_pv(*w[:4])
            pend.clear()
            while norm_q:
                norm_q.pop(0)()

    _split_matmul_waits(nc, mybir)
    return nc


def _split_matmul_waits(nc, mybir):
    """walrus's codegen only has room for one sync-wait in the engine
    micro-op structs; peel extra waits off onto wait-only EventSemaphore
    instructions on the same engine queue just before.

    First, drop waits that are trivially satisfied: a sem-ge wait on a
    semaphore that is only ever incremented by instructions on this same
    (in-order, FIFO-completing) engine queue is redundant -- by the time
    this instruction dispatches, all its predecessors have completed."""
    skip = (mybir.InstEventSemaphore,)
    sem_engines = {}
    sem_clean = {}
    for bb in nc.main_func.blocks:
        for ins in bb.instructions:
            si = ins.sync_info
            if si is None or not si.on_update:
                continue
            for up in si.on_update:
                sem_engines.setdefault(up.id, set()).add(str(ins.engine))
                ok = (
                    up.update_mode == "sem-inc"
                    and up.update_reg is None
                    and "DMA" not in type(ins).__name__
                )
                sem_clean[up.id] = sem_clean.get(up.id, True) and ok

    def is_redundant(ins, wait):
        return (
            wait.wait_mode == "sem-ge-imm"
            and wait.wait_reg is None
            and sem_clean.get(wait.id, False)
            and sem_engines.get(wait.id) == {str(ins.engine)}
        )

    for bb in nc.main_func.blocks:
        for ins in bb.instructions:
            if isinstance(ins, skip):
                continue
            si = ins.sync_info
            if si is not None and si.on_wait and len(si.on_wait) > 1:
                kept = [w for w in si.on_wait if not is_redundant(ins, w)]
                if len(kept) != len(si.on_wait):
                    if not kept:
                        kept = [si.on_wait[-1]]
                    ins.sync_info = mybir.SyncInfo(
                        on_wait=kept, on_update=list(si.on_update or [])
                    )
    for bb in nc.main_func.blocks:
        insts = list(bb.instructions)
        out = []
        changed = False
        for ins in insts:
            if not isinstance(ins, skip):
                si = ins.sync_info
                if si is not None and si.on_wait and len(si.on_wait) > 1:
                    for wi, wait in enumerate(si.on_wait[:-1]):
                        w = mybir.InstEventSemaphore(
                            name=f"{ins.name}_prewait{wi}"
                        )
                        w.engine = ins.engine
                        w.sync_info = mybir.SyncInfo(
                            on_wait=[wait], on_update=[]
                        )
                        out.append(w)
                    ins.sync_info = mybir.SyncInfo(
                        on_wait=[si.on_wait[-1]],
                        on_update=list(si.on_update or []),
                    )
                    changed = True
            out.append(ins)
        if changed:
            bb.instructions = out


def _prep_inputs(x, w_qkv, b_qkv, w_out, b_out):
    """Host-side input prep -> per-core in_maps."""
    import ml_dtypes

    x = np.ascontiguousarray(np.asarray(x, dtype=np.float32))
    w_qkv = np.asarray(w_qkv, dtype=np.float32)
    b_qkv = np.asarray(b_qkv, dtype=np.float32)
    w_out = np.asarray(w_out, dtype=np.float32)
    b_out = np.asarray(b_out, dtype=np.float32)

    scale = 1.0 / np.sqrt(np.float32(C))
    wq = np.concatenate([w_qkv[0:C].T, b_qkv[None, 0:C]], axis=0) * scale
    wk = np.concatenate([w_qkv[C:2 * C].T, b_qkv[None, C:2 * C]], axis=0)
    wv = np.concatenate([w_qkv[2 * C:].T, b_qkv[None, 2 * C:]], axis=0)
    wo = np.ascontiguousarray(w_out.T)
    wq = np.ascontiguousarray(wq, dtype=np.float32)
    wk = np.ascontiguousarray(wk, dtype=np.float32)
    wv = np.ascontiguousarray(wv, dtype=np.float32)

    F8 = ml_dtypes.float8_e4m3
    cq1 = np.zeros((128, HW), dtype=F8)
    cq1[C, :] = F8(1.0)          # bias channel: q side = 1.0
    ck1 = np.zeros((128, HW), dtype=F8)
    ck1[C, :] = F8(SBIAS)        # bias channel: k side = +2.0 score offset

    ones = np.ones((1, HW), dtype=np.float32)
    in_maps = []
    for b in range(B):
        xcm = x[b].reshape(C, HW)
        xa = np.concatenate([xcm, ones], axis=0)
        xb = xcm + b_out[:, None].astype(np.float32)
        in_maps.append(
            {
                "xa": np.ascontiguousarray(xa),
                "xb": np.ascontiguousarray(xb),
                "wq": wq,
                "wk": wk,
                "wv": wv,
                "wo": wo,
                "cq1": cq1,
                "ck1": ck1,
            }
        )
    return in_maps


def _get_compiled():
    global _compiled
    if _compiled is None:
        _compiled = _build_bass()
    return _compiled


def kernel(x, w_qkv, b_qkv, w_out, b_out, _trace=False, _trace_kwargs=None):
    from concourse.bass_utils import run_bass_kernel_spmd

    nc = _get_compiled()
    in_maps = _prep_inputs(x, w_qkv, b_qkv, w_out, b_out)
    res = run_bass_kernel_spmd(
        nc,
        in_maps,
        list(range(NCORES)),
        trace=_trace,
        **(_trace_kwargs or {}),
    )
    out = np.stack([res.results[b]["out"].reshape(C, 64, 64) for b in range(B)])
    if _trace:
        kernel._last_results = res
    return out.astype(np.float32)

